# revision 1
# baseline (speedup 1.0000x reference)
"""GAT-style GNN message passing on 8 TRN2 NeuronCores.

Math: with LEAK=1 the leaky-relu is identity, so
  e[i,j,h] = e_src[i,h] + e_dst[j,h]
and softmax over j cancels e_src (and any row max) exactly:
  attn[i,j,h] = adj[i,j]*exp(e_dst[j,h]) / sum_j adj[i,j]*exp(e_dst[j,h])
  out[i,(h,f)] = (adj @ (z*h))[i,(h,f)] / (adj @ z)[i,h],  z = exp(e_dst)
then elu + log_softmax per row. log_softmax is shift invariant, so
elu(x) is computed as relu(x) + exp(min(x,0)) (drops the uniform -1),
and no max subtraction is needed (y is bounded in [e^-10, ~10]).

Sharding: rows (query nodes) of adj/out across 8 cores. x is row-sharded
too; each core computes its local h slab, all-gathers G=[z*h | z],
then computes its [N/8, 64] output slab locally.

The aggregation matmul adj @ G runs in bf16 at full PE rate but stays
EXACT to ~2^-16: adj entries are 0/1 (exact in bf16) and G is sent as a
bf16 hi/lo split (G = hi + lo, two accumulating matmuls into fp32 PSUM)
— same bytes as fp32, half the PE cycles of the fp32 4-cycle/row mode,
and no bf16->fp32 cast pass over the 4MB adjacency.

All DRAM<->SBUF tensors use partition-major host layouts ([128, ...],
one contiguous run per partition) so each DMA needs ~128 descriptors
(~3.5ns/descriptor on the HWDGE queue otherwise dominates).

Per-core device program (R = N/8 = 512 rows, P=128):
  inputs:  xt [128, KC*R] f32   xt[p, kc*R+r]  = x[c*R+r, kc*128+p]
           wt [128, KC*72] f32  wt[p, kc*72+e] = w_ext[kc*128+p, e]
                                (w_ext = [W | W @ blockdiag-reduced a_dst])
           at [128, NC*R] bf16  at[p, n*R+r]   = adj[c*R+r, n*128+p]
  output:  out_p [128, RC*64]   out_p[p, q*64+f] = out[q*128+p, f]
"""

import sys

import numpy as np

if "/opt/trn_rl_repo" not in sys.path:
    sys.path.insert(0, "/opt/trn_rl_repo")

import ml_dtypes  # noqa: E402

import concourse.bass as bass  # noqa: E402
import concourse.tile as tile  # noqa: E402
from concourse import bacc, mybir  # noqa: E402
from concourse.bass_utils import run_bass_kernel_spmd  # noqa: E402
from concourse.masks import make_identity  # noqa: E402

N_CORES = 8
H = 8
F = 8
HF = H * F  # 64
EXT = HF + H  # 72: [g | z]
K_IN = 1024
P = 128

FP32 = mybir.dt.float32
BF16 = mybir.dt.bfloat16
AFT = mybir.ActivationFunctionType
ALU = mybir.AluOpType


def _bcast_head(ap_ph):
    """[P, H] AP -> [P, H, F] AP broadcasting each head value over F."""
    return bass.AP(
        tensor=ap_ph.tensor,
        offset=ap_ph.offset,
        ap=[ap_ph.ap[0], ap_ph.ap[1], [0, F]],
    )


def build_bass(n_nodes: int) -> bass.Bass:
    R = n_nodes // N_CORES
    KC = K_IN // P  # k-chunks for the h matmul
    NC = n_nodes // P  # j-chunks for the aggregation matmul
    RC = R // P  # 128-row output chunks per core
    assert R % P == 0

    # Bacc (not plain Bass): its finalize() runs move_matmul_waits_to_ldweights
    # + generate_event_semaphores, which legalize multi-wait instructions for
    # walrus (TRN2 allows at most 1 sync wait per instruction).
    nc = bacc.Bacc(num_devices=N_CORES)

    xt = nc.declare_dram_parameter("xt", [P, KC * R], FP32, isOutput=False)
    at = nc.declare_dram_parameter("at", [P, NC * R], BF16, isOutput=False)
    wt = nc.declare_dram_parameter("wt", [P, KC * EXT], FP32, isOutput=False)
    out = nc.declare_dram_parameter("out", [P, RC * HF], FP32, isOutput=True)

    # DRAM collectives concatenate the ranks' buffers FLAT (block-major).
    # G is gathered in two pipelined halves (q-chunks 0..RC/2-1, RC/2..RC-1)
    # so the second AllGather's mesh overlaps the first half's matmuls.
    HB = RC // 2  # q-chunks per half
    g_loc_a = nc.dram_tensor("g_loc_a", [P, HB * 2 * EXT], BF16)
    g_loc_b = nc.dram_tensor("g_loc_b", [P, HB * 2 * EXT], BF16)
    g_full_a = nc.dram_tensor(
        "g_full_a", [N_CORES, P, HB * 2 * EXT], BF16, addr_space="Shared"
    )
    g_full_b = nc.dram_tensor(
        "g_full_b", [N_CORES, P, HB * 2 * EXT], BF16, addr_space="Shared"
    )

    with tile.TileContext(nc) as tc:
        with (
            tc.tile_pool(name="singles", bufs=1) as singles,
            tc.tile_pool(name="bigpsum", bufs=2, space="PSUM") as bigpsum,
            tc.tile_pool(name="smallpsum", bufs=4, space="PSUM") as smallpsum,
            tc.tile_pool(name="work", bufs=4) as work,
            tc.tile_pool(name="post", bufs=4) as post,
        ):
            ident = singles.tile([P, P], FP32)
            make_identity(nc, ident)

            # --- loads (p-major, one run per partition) ---
            w_sb = singles.tile([P, KC, EXT], FP32)
            nc.sync.dma_start(
                out=w_sb, in_=wt[:].rearrange("p (c e) -> p c e", c=KC)
            )
            xt_sb = singles.tile([P, KC, R], FP32)
            xt_view = xt[:].rearrange("p (c r) -> p c r", c=KC)
            nc.sync.dma_start(out=xt_sb[:, : KC // 2, :], in_=xt_view[:, : KC // 2, :])
            nc.sync.dma_start(out=xt_sb[:, KC // 2 :, :], in_=xt_view[:, KC // 2 :, :])

            # --- hT = w_ext.T @ x_loc.T : [EXT, R] (fp32, exact), computed
            # in two column halves so the first half's transposes + AllGather
            # trigger before the second half's matmuls finish. ---
            hT_sb = singles.tile([EXT, R], FP32)
            RH = R // 2
            for half in range(2):
                hT_ps = bigpsum.tile([EXT, RH], FP32, tag="bigps", name=f"hT{half}")
                cols = slice(half * RH, (half + 1) * RH)
                for c in range(KC):
                    nc.tensor.matmul(
                        hT_ps,
                        lhsT=w_sb[:, c, :],
                        rhs=xt_sb[:, c, cols],
                        start=(c == 0),
                        stop=(c == KC - 1),
                    )
                nc.vector.tensor_copy(hT_sb[:, cols], hT_ps)

            # --- per 128-chunk: transpose, z=exp, G=[h*z | z], hi/lo bf16 ---
            ghl_sb = singles.tile([P, RC, 2, EXT], BF16)
            for q in range(RC):
                h_ps = smallpsum.tile([P, EXT], FP32, tag="smallps")
                nc.tensor.transpose(
                    h_ps, hT_sb[:, q * P : (q + 1) * P], ident[:EXT, :EXT]
                )
                g_sb = work.tile([P, EXT], FP32, tag="g")
                z_sb = work.tile([P, H], FP32, tag="z")
                nc.scalar.activation(z_sb, h_ps[:, HF:EXT], AFT.Exp)
                nc.vector.tensor_mul(
                    g_sb[:, 0:HF].rearrange("p (h f) -> p h f", h=H),
                    h_ps[:, 0:HF].rearrange("p (h f) -> p h f", h=H),
                    _bcast_head(z_sb),
                )
                nc.vector.tensor_copy(g_sb[:, HF:EXT], z_sb)
                # hi/lo split: exact bf16 representation of fp32 G
                nc.vector.tensor_copy(ghl_sb[:, q, 0, :], g_sb)
                lo_sb = work.tile([P, EXT], FP32, tag="lo")
                nc.vector.tensor_copy(lo_sb, ghl_sb[:, q, 0, :])
                nc.vector.tensor_sub(lo_sb, g_sb, lo_sb)
                nc.vector.tensor_copy(ghl_sb[:, q, 1, :], lo_sb)
                if q == HB - 1:
                    nc.sync.dma_start(out=g_loc_a[:], in_=ghl_sb[:, :HB])
                    nc.gpsimd.collective_compute(
                        "AllGather",
                        ALU.bypass,
                        replica_groups=[list(range(N_CORES))],
                        ins=[g_loc_a[:]],
                        outs=[g_full_a[:]],
                    )
                elif q == RC - 1:
                    nc.sync.dma_start(out=g_loc_b[:], in_=ghl_sb[:, HB:])
                    nc.gpsimd.collective_compute(
                        "AllGather",
                        ALU.bypass,
                        replica_groups=[list(range(N_CORES))],
                        ins=[g_loc_b[:]],
                        outs=[g_full_b[:]],
                    )

            # --- adjT load (bf16, consumed directly by the PE) ---
            at_sb = singles.tile([P, NC, R], BF16)
            at_view = at[:].rearrange("p (n r) -> p n r", n=NC)
            N_SPLITS = 4
            for s in range(N_SPLITS):
                lo, hi = NC // N_SPLITS * s, NC // N_SPLITS * (s + 1)
                nc.sync.dma_start(out=at_sb[:, lo:hi, :], in_=at_view[:, lo:hi, :])

            # --- load gathered G halves, aggregate: outT += G_n.T @ adjT_n ---
            # g_all_X[p, c, q2, s, e] = (hi,lo)[s] of G[c*R + (q2+off)*128 + p, e]
            g_all_a = singles.tile([P, N_CORES, HB, 2, EXT], BF16)
    
            g_all_b = singles.tile([P, N_CORES, HB, 2, EXT], BF16)
            gfa_view = g_full_a[:].rearrange("c p (q s e) -> p c q s e", q=HB, s=2)
            gfb_view = g_full_b[:].rearrange("c p (q s e) -> p c q s e", q=HB, s=2)
            for s in range(2):
                lo, hi = N_CORES // 2 * s, N_CORES // 2 * (s + 1)
                nc.sync.dma_start(out=g_all_a[:, lo:hi], in_=gfa_view[:, lo:hi])
            for s in range(2):
                lo, hi = N_CORES // 2 * s, N_CORES // 2 * (s + 1)
                nc.sync.dma_start(out=g_all_b[:, lo:hi], in_=gfb_view[:, lo:hi])
            outT_ps = bigpsum.tile([EXT, R], FP32, tag="bigps")
            first = True
            for half, g_all_h, qoff in ((0, g_all_a, 0), (1, g_all_b, HB)):
                for c in range(N_CORES):
                    for q2 in range(HB):
                        n = c * RC + qoff + q2
                        for s in range(2):
                            nc.tensor.matmul(
                                outT_ps,
                                lhsT=g_all_h[:, c, q2, s, :],
                                rhs=at_sb[:, n, :],
                                start=first,
                                stop=(half == 1 and c == N_CORES - 1
                                      and q2 == HB - 1 and s == 1),
                            )
                            first = False
            outT_sb = singles.tile([EXT, R], FP32)
            nc.vector.tensor_copy(outT_sb, outT_ps)

            # --- postprocess, batched per stage across the RC chunks ---
            o_ps = [None] * RC
            for q in range(RC):
                o_ps[q] = smallpsum.tile([P, EXT], FP32, tag="smallps", name=f"o_ps{q}")
                nc.tensor.transpose(
                    o_ps[q], outT_sb[:, q * P : (q + 1) * P], ident[:EXT, :EXT]
                )
            xo = [None] * RC
            for q in range(RC):
                rd = work.tile([P, H], FP32, tag="rd")
                nc.vector.reciprocal(rd, o_ps[q][:, HF:EXT])
                xo[q] = post.tile([P, HF], FP32, tag="xo", name=f"xo{q}")
                nc.vector.tensor_mul(
                    xo[q].rearrange("p (h f) -> p h f", h=H),
                    o_ps[q][:, 0:HF].rearrange("p (h f) -> p h f", h=H),
                    _bcast_head(rd),
                )
            # y = relu(xo) + exp(min(xo, 0))  (= elu + 1; log_softmax shift-safe)
            yo = [None] * RC
            eo = [None] * RC
            for q in range(RC):
                mo = work.tile([P, HF], FP32, tag="mo")
                nc.vector.tensor_scalar_min(mo, xo[q], 0.0)
                eo[q] = post.tile([P, HF], FP32, tag="eo", name=f"eo{q}")
                nc.scalar.activation(eo[q], mo, AFT.Exp)
            for q in range(RC):
                yo[q] = post.tile([P, HF], FP32, tag="yo", name=f"yo{q}")
                nc.vector.scalar_tensor_tensor(
                    out=yo[q], in0=xo[q], scalar=0.0, in1=eo[q],
                    op0=ALU.max, op1=ALU.add,
                )
            # log-softmax over the 64 features (no max subtraction needed:
            # y in (0, ~10], exp stays in fp32 range); batch Exp then Ln to
            # avoid ACT table-set thrash.
            ex = [None] * RC
            sm = [None] * RC
            for q in range(RC):
                ex[q] = post.tile([P, HF], FP32, tag="ex", name=f"ex{q}")
                nc.scalar.activation(ex[q], yo[q], AFT.Exp)
            for q in range(RC):
                sm[q] = post.tile([P, 1], FP32, tag="sm", name=f"sm{q}")
                nc.vector.reduce_sum(sm[q], ex[q], axis=mybir.AxisListType.X)
            out_sb = singles.tile([P, RC, HF], FP32)
            for q in range(RC):
                ls = work.tile([P, 1], FP32, tag="ls")
                nc.scalar.activation(ls, sm[q], AFT.Ln)
                nc.vector.tensor_scalar_sub(out_sb[:, q, :], yo[q], ls)
            nc.sync.dma_start(out=out[:], in_=out_sb)

    # Force all ACT activations (Exp + Ln) onto the one table set containing
    # both, so only ONE ACT_TABLE_LOAD is emitted (early, hidden under DMA)
    # instead of a ~1.3us reload at every Exp<->Ln switch. Set indices must
    # stay aligned with act_info.json, so empty the other sets rather than
    # filtering the list.
    orig_gat = bacc.get_activation_tables

    def _one_set(arch):
        return {
            k: (v if k == "natural_log_exp_and_others" else set())
            for k, v in orig_gat(arch).items()
        }

    bacc.get_activation_tables = _one_set
    try:
        nc.finalize()
    finally:
        bacc.get_activation_tables = orig_gat
    return nc


def _pmajor(a, chunk):
    """[chunk*P, L] -> [P, chunk*L] partition-major layout."""
    n, L = a.shape[0] // P, a.shape[1]
    return np.ascontiguousarray(
        a.reshape(n, P, L).transpose(1, 0, 2).reshape(P, n * L)
    )


def _host_prep(x, adj, W, a_dst, n_nodes):
    """Build per-core input maps."""
    R = n_nodes // N_CORES
    Wd = np.einsum(
        "khf,hf->kh", W.reshape(K_IN, H, F), a_dst, dtype=np.float32
    ).astype(np.float32)
    w_ext = np.concatenate([W, Wd], axis=1).astype(np.float32)  # [1024, 72]
    wt = _pmajor(w_ext, K_IN // P)
    adj_bf = adj.astype(ml_dtypes.bfloat16)  # exact for 0/1
    in_maps = []
    for c in range(N_CORES):
        rows = slice(c * R, (c + 1) * R)
        in_maps.append(
            {
                "xt": _pmajor(np.ascontiguousarray(x[rows].T.astype(np.float32)), K_IN // P),
                "at": _pmajor(np.ascontiguousarray(adj_bf[rows].T), n_nodes // P),
                "wt": wt,
            }
        )
    return in_maps


_BUILT = {}


def run(x, adj, W, a_dst, trace=False):
    n_nodes = x.shape[0]
    R = n_nodes // N_CORES
    RC = R // P
    if n_nodes not in _BUILT:
        _BUILT[n_nodes] = build_bass(n_nodes)
    nc = _BUILT[n_nodes]
    in_maps = _host_prep(x, adj, W, a_dst, n_nodes)
    res = run_bass_kernel_spmd(
        nc, in_maps, list(range(N_CORES)), trace=trace
    )
    blocks = []
    for c in range(N_CORES):
        o = res.results[c]["out"]  # [P, RC*HF] p-major
        blocks.append(
            o.reshape(P, RC, HF).transpose(1, 0, 2).reshape(R, HF)
        )
    return np.concatenate(blocks, axis=0).astype(np.float32), res


def kernel(x, adj, W, a_src, a_dst):
    x = np.asarray(x, dtype=np.float32)
    adj = np.asarray(adj)
    W = np.asarray(W, dtype=np.float32)
    a_dst = np.asarray(a_dst, dtype=np.float32)
    out, _ = run(x, adj, W, a_dst, trace=False)
    return out



# revision 3
# speedup vs baseline: 2.0603x; 2.0603x over previous
"""GAT-style GNN message passing on 8 TRN2 NeuronCores — collective-free.

Math: with LEAK=1 the leaky-relu is identity, so
  e[i,j,h] = e_src[i,h] + e_dst[j,h]
and softmax over j cancels e_src (and any row max) exactly:
  attn[i,j,h] = adj[i,j]*exp(e_dst[j,h]) / sum_j adj[i,j]*exp(e_dst[j,h])
  out[i,(h,f)] = (adj @ (z*h))[i,(h,f)] / (adj @ z)[i,h],  z = exp(e_dst)
then elu + log_softmax per row. log_softmax is shift invariant, so
elu(x) is computed as relu(x) + exp(min(x,0)) (drops the uniform -1),
and no max subtraction is needed (y is bounded in (0, ~10]).

Sharding: rows (query nodes) of adj/out across 8 cores. The h = x@W
computation (cheap: 0.6 GFLOP) is REPLICATED on every core from a full
copy of x, so there are no collectives at all: a previous version
all-gathered h and paid a ~40us CC barrier + ~28us of AllGather that
dominated the runtime. Replication costs +6MB of HBM reads per core
but removes every cross-core dependency (no barrier, no launch-skew
sensitivity, PE stays HAM-warm).

Dtypes: x/W in bf16 (h error ~0.5% of h std -> ~2e-4 final rel err),
adjacency in fp8e4 (0/1 exact, half the DMA bytes of bf16), G=[z*h|z]
in bf16 as the stationary matmul operand against the fp8 moving adjT
(mixed non-fp32 operand dtypes are legal on the PE). Aggregation
accumulates in fp32 PSUM.

All DRAM->SBUF tensors use partition-major host layouts ([128, ...],
one contiguous run per partition) so each DMA needs ~128 descriptors.

Per-core device program (R = N/8 = 512 rows, P = 128):
  inputs:  xt [128, KC*N]  bf16  xt[p, kc*N+n]   = x[n, kc*128+p]
           wt [128, KC*72] bf16  wt[p, kc*72+e]  = w_ext[kc*128+p, e]
                                 (w_ext = [W | W @ blockdiag-reduced a_dst])
           at [128, NJ*R]  fp8   at[p, nj*R+r]   = adj[c*R+r, nj*128+p]
  output:  out_p [128, RC*64] f32  out_p[p, q*64+f] = out[c*R+q*128+p, (h,f)]
"""

import sys

import numpy as np

if "/opt/trn_rl_repo" not in sys.path:
    sys.path.insert(0, "/opt/trn_rl_repo")

import ml_dtypes  # noqa: E402

import concourse.bass as bass  # noqa: E402
import concourse.tile as tile  # noqa: E402
from concourse import bacc, mybir  # noqa: E402
from concourse.bass_utils import run_bass_kernel_spmd  # noqa: E402
from concourse.masks import make_identity  # noqa: E402

N_CORES = 8
H = 8
F = 8
HF = H * F  # 64
EXT = HF + H  # 72: [g | z]
K_IN = 1024
P = 128

FP32 = mybir.dt.float32
BF16 = mybir.dt.bfloat16
FP8 = mybir.dt.float8e4
AFT = mybir.ActivationFunctionType
ALU = mybir.AluOpType


def _bcast_head(ap_ph):
    """[P, H] AP -> [P, H, F] AP broadcasting each head value over F."""
    return bass.AP(
        tensor=ap_ph.tensor,
        offset=ap_ph.offset,
        ap=[ap_ph.ap[0], ap_ph.ap[1], [0, F]],
    )


def build_bass(n_nodes: int) -> bass.Bass:
    R = n_nodes // N_CORES
    KC = K_IN // P  # k-chunks for the h matmul
    NJ = n_nodes // P  # j (neighbor) 128-chunks
    RC = R // P  # 128-row output chunks per core
    NHALF = n_nodes // 2  # node-axis halves for the h pipeline
    GH = NHALF // 512  # 512-col psum groups per half (4)
    QH = NHALF // P  # 128-node chunks per half (16)
    assert R % P == 0

    # Bacc finalize() runs move_matmul_waits_to_ldweights +
    # generate_event_semaphores, which legalize multi-wait instructions
    # for walrus (TRN2 allows at most 1 sync wait per instruction).
    nc = bacc.Bacc(num_devices=N_CORES)

    xt = nc.declare_dram_parameter("xt", [P, KC * n_nodes], BF16, isOutput=False)
    wt = nc.declare_dram_parameter("wt", [P, KC * EXT], BF16, isOutput=False)
    at = nc.declare_dram_parameter("at", [P, NJ * R], FP8, isOutput=False)
    out = nc.declare_dram_parameter("out", [P, RC * HF], FP32, isOutput=True)

    with tile.TileContext(nc) as tc:
        with (
            tc.tile_pool(name="singles", bufs=1) as singles,
            tc.tile_pool(name="xstream", bufs=4) as xstream,
            tc.tile_pool(name="bigpsum", bufs=4, space="PSUM") as bigpsum,
            tc.tile_pool(name="smallpsum", bufs=4, space="PSUM") as smallpsum,
            tc.tile_pool(name="work", bufs=4) as work,
            tc.tile_pool(name="post", bufs=4) as post,
        ):
            ident = singles.tile([P, P], FP32)
            make_identity(nc, ident)

            # --- loads (p-major, one contiguous run per partition) ---
            w_sb = singles.tile([P, KC, EXT], BF16)
            nc.sync.dma_start(
                out=w_sb, in_=wt[:].rearrange("p (c e) -> p c e", c=KC)
            )
            xt_view = xt[:].rearrange("p (c n) -> p c n", c=KC)

            hT_sb = singles.tile([EXT, n_nodes], FP32)
            g_sb = singles.tile([P, NJ, EXT], BF16)

            # --- hT = w_ext.T @ x.T in two node-halves, kc-outer so the
            # matmuls stream behind the xt DMA chunks; the half-A
            # transposes/z/g overlap half B's DMA+matmuls. ---
            for half in range(2):
                noff = half * NHALF
                hps = [
                    bigpsum.tile([EXT, 512], FP32, tag="bigps", name=f"h{half}_{g}")
                    for g in range(GH)
                ]
                for kc in range(KC):
                    xbuf = xstream.tile([P, NHALF], BF16, tag="xs")
                    nc.sync.dma_start(
                        out=xbuf, in_=xt_view[:, kc, noff : noff + NHALF]
                    )
                    for g in range(GH):
                        nc.tensor.matmul(
                            hps[g],
                            lhsT=w_sb[:, kc, :],
                            rhs=xbuf[:, g * 512 : (g + 1) * 512],
                            start=(kc == 0),
                            stop=(kc == KC - 1),
                        )
                for g in range(GH):
                    nc.vector.tensor_copy(
                        hT_sb[:, noff + g * 512 : noff + (g + 1) * 512], hps[g]
                    )

                # --- per 128-node chunk: transpose, z=exp, G=[h*z | z] ---
                for q2 in range(QH):
                    q = half * QH + q2
                    h_ps = smallpsum.tile([P, EXT], FP32, tag="smallps")
                    nc.tensor.transpose(
                        h_ps, hT_sb[:, q * P : (q + 1) * P], ident[:EXT, :EXT]
                    )
                    z_sb = work.tile([P, H], FP32, tag="z")
                    nc.scalar.activation(z_sb, h_ps[:, HF:EXT], AFT.Exp)
                    nc.vector.tensor_mul(
                        g_sb[:, q, 0:HF].rearrange("p (h f) -> p h f", h=H),
                        h_ps[:, 0:HF].rearrange("p (h f) -> p h f", h=H),
                        _bcast_head(z_sb),
                    )
                    nc.vector.tensor_copy(g_sb[:, q, HF:EXT], z_sb)

            # --- adjT load (fp8, consumed directly by the PE) ---
            at_sb = singles.tile([P, NJ, R], FP8)
            at_view = at[:].rearrange("p (n r) -> p n r", n=NJ)
            N_SPLITS = 4
            for s in range(N_SPLITS):
                lo, hi = NJ // N_SPLITS * s, NJ // N_SPLITS * (s + 1)
                nc.sync.dma_start(out=at_sb[:, lo:hi, :], in_=at_view[:, lo:hi, :])

            # --- aggregate: outT += G_n.T @ adjT_n over all 32 j-chunks ---
            outT_ps = bigpsum.tile([EXT, R], FP32, tag="bigps", name="outT")
            for n in range(NJ):
                nc.tensor.matmul(
                    outT_ps,
                    lhsT=g_sb[:, n, :],
                    rhs=at_sb[:, n, :],
                    start=(n == 0),
                    stop=(n == NJ - 1),
                )
            outT_sb = singles.tile([EXT, R], FP32)
            nc.vector.tensor_copy(outT_sb, outT_ps)

            # --- postprocess, batched per stage across the RC chunks ---
            o_ps = [None] * RC
            for q in range(RC):
                o_ps[q] = smallpsum.tile([P, EXT], FP32, tag="smallps", name=f"o{q}")
                nc.tensor.transpose(
                    o_ps[q], outT_sb[:, q * P : (q + 1) * P], ident[:EXT, :EXT]
                )
            xo = [None] * RC
            for q in range(RC):
                rd = work.tile([P, H], FP32, tag="rd")
                nc.vector.reciprocal(rd, o_ps[q][:, HF:EXT])
                xo[q] = post.tile([P, HF], FP32, tag="xo", name=f"xo{q}")
                nc.vector.tensor_mul(
                    xo[q].rearrange("p (h f) -> p h f", h=H),
                    o_ps[q][:, 0:HF].rearrange("p (h f) -> p h f", h=H),
                    _bcast_head(rd),
                )
            # y = relu(xo) + exp(min(xo, 0))  (= elu + 1; log_softmax shift-safe)
            yo = [None] * RC
            eo = [None] * RC
            for q in range(RC):
                mo = work.tile([P, HF], FP32, tag="mo")
                nc.vector.tensor_scalar_min(mo, xo[q], 0.0)
                eo[q] = post.tile([P, HF], FP32, tag="eo", name=f"eo{q}")
                nc.scalar.activation(eo[q], mo, AFT.Exp)
            for q in range(RC):
                yo[q] = post.tile([P, HF], FP32, tag="yo", name=f"yo{q}")
                nc.vector.scalar_tensor_tensor(
                    out=yo[q], in0=xo[q], scalar=0.0, in1=eo[q],
                    op0=ALU.max, op1=ALU.add,
                )
            # log-softmax over the 64 features (no max subtraction needed:
            # y in (0, ~10], exp stays in fp32 range); batch Exp then Ln to
            # avoid ACT table-set thrash.
            ex = [None] * RC
            sm = [None] * RC
            for q in range(RC):
                ex[q] = post.tile([P, HF], FP32, tag="ex", name=f"ex{q}")
                nc.scalar.activation(ex[q], yo[q], AFT.Exp)
            for q in range(RC):
                sm[q] = post.tile([P, 1], FP32, tag="sm", name=f"sm{q}")
                nc.vector.reduce_sum(sm[q], ex[q], axis=mybir.AxisListType.X)
            out_sb = singles.tile([P, RC, HF], FP32)
            for q in range(RC):
                ls = work.tile([P, 1], FP32, tag="ls")
                nc.scalar.activation(ls, sm[q], AFT.Ln)
                nc.vector.tensor_scalar_sub(out_sb[:, q, :], yo[q], ls)
            nc.sync.dma_start(out=out[:], in_=out_sb)

    # Force all ACT activations (Exp + Ln) onto the one table set containing
    # both, so only ONE ACT_TABLE_LOAD is emitted (early, hidden under DMA)
    # instead of a ~1.3us reload at every Exp<->Ln switch. Set indices must
    # stay aligned with act_info.json, so empty the other sets rather than
    # filtering the list.
    orig_gat = bacc.get_activation_tables

    def _one_set(arch):
        return {
            k: (v if k == "natural_log_exp_and_others" else set())
            for k, v in orig_gat(arch).items()
        }

    bacc.get_activation_tables = _one_set
    try:
        nc.finalize()
    finally:
        bacc.get_activation_tables = orig_gat
    return nc


def _host_prep(x, adj, W, a_dst, n_nodes):
    """Build per-core input maps."""
    R = n_nodes // N_CORES
    NJ = n_nodes // P
    Wd = np.einsum(
        "khf,hf->kh", W.reshape(K_IN, H, F), a_dst, dtype=np.float32
    ).astype(np.float32)
    w_ext = np.concatenate([W, Wd], axis=1)  # [1024, 72] fp32
    # wt[p, kc*72+e] = w_ext[kc*128+p, e]
    wt = np.ascontiguousarray(
        w_ext.reshape(KC := K_IN // P, P, EXT).transpose(1, 0, 2).reshape(P, KC * EXT)
    ).astype(ml_dtypes.bfloat16)
    # xt[p, kc*N+n] = x[n, kc*128+p]  (shared by all cores)
    xt = np.ascontiguousarray(
        x.astype(ml_dtypes.bfloat16).T.reshape(KC, P, n_nodes).transpose(1, 0, 2)
        .reshape(P, KC * n_nodes)
    )
    # adj as fp8e4: 0/1 exact. Byte-level cast (1.0 == 0x38) beats .astype.
    adj_u8 = (adj.astype(np.uint8) * np.uint8(0x38)).view(ml_dtypes.float8_e4m3)
    in_maps = []
    for c in range(N_CORES):
        rows = slice(c * R, (c + 1) * R)
        # at[p, nj*R+r] = adj[c*R+r, nj*128+p]
        at = np.ascontiguousarray(
            adj_u8[rows].T.reshape(NJ, P, R).transpose(1, 0, 2).reshape(P, NJ * R)
        )
        in_maps.append({"xt": xt, "at": at, "wt": wt})
    return in_maps


_BUILT = {}


def run(x, adj, W, a_dst, trace=False):
    n_nodes = x.shape[0]
    R = n_nodes // N_CORES
    RC = R // P
    if n_nodes not in _BUILT:
        _BUILT[n_nodes] = build_bass(n_nodes)
    nc = _BUILT[n_nodes]
    in_maps = _host_prep(x, adj, W, a_dst, n_nodes)
    res = run_bass_kernel_spmd(
        nc, in_maps, list(range(N_CORES)), trace=trace
    )
    blocks = []
    for c in range(N_CORES):
        o = res.results[c]["out"]  # [P, RC*HF] p-major
        blocks.append(
            o.reshape(P, RC, HF).transpose(1, 0, 2).reshape(R, HF)
        )
    return np.concatenate(blocks, axis=0).astype(np.float32), res


def kernel(x, adj, W, a_src, a_dst):
    x = np.asarray(x, dtype=np.float32)
    adj = np.asarray(adj)
    W = np.asarray(W, dtype=np.float32)
    a_dst = np.asarray(a_dst, dtype=np.float32)
    out, _ = run(x, adj, W, a_dst, trace=False)
    return out


# revision 4
# speedup vs baseline: 2.1860x; 1.0610x over previous
"""GAT-style GNN message passing on 8 TRN2 NeuronCores — collective-free.

Math: with LEAK=1 the leaky-relu is identity, so
  e[i,j,h] = e_src[i,h] + e_dst[j,h]
and softmax over j cancels e_src (and any row max) exactly:
  attn[i,j,h] = adj[i,j]*exp(e_dst[j,h]) / sum_j adj[i,j]*exp(e_dst[j,h])
  out[i,(h,f)] = (adj @ (z*h))[i,(h,f)] / (adj @ z)[i,h],  z = exp(e_dst)
then elu + log_softmax per row. log_softmax is shift invariant, so
elu(x) is computed as relu(x) + exp(min(x,0)) (drops the uniform -1),
and no max subtraction is needed (y is bounded in (0, ~10]).

Sharding: rows (query nodes) of adj/out across 8 cores. The h = x@W
computation (cheap: 0.6 GFLOP) is REPLICATED on every core from a full
copy of x, so there are no collectives at all: a previous version
all-gathered h and paid a ~40us CC barrier + ~28us of AllGather that
dominated the runtime. Replication costs extra HBM reads per core but
removes every cross-core dependency (no barrier, no launch-skew
sensitivity, PE stays HAM-warm).

Dtypes (HW runs are HBM-bandwidth bound, so bytes are everything):
  x in fp8e4 (4MB replicated; ~1e-3 final rel err, gate is 2e-2),
  W in bf16 (mixed bf16-stationary x fp8-moving matmul is legal and
  was verified exact against the numpy model on HW),
  adjacency in fp8e4 (0/1 exact, 2MB per core),
  G=[z*h | z] in bf16 stationary against the fp8 moving adjT.
Aggregation accumulates in fp32 PSUM.

Pipeline: node axis in two halves. Per half: stream xt k-chunks (DMA)
-> h matmuls into 4 psum banks (kc-outer) -> evacuate -> per-128-node
transpose + z/g. The adjacency for half A's j-chunks loads right after
half A's xt, so the first 16 aggregation matmuls run while half B is
still streaming. Order on the DMA queue: w, xtA, atA, xtB, atB.

All DRAM->SBUF tensors use partition-major host layouts ([128, ...],
one contiguous run per partition).

Per-core device program (R = N/8 = 512 rows, P = 128):
  inputs:  xt [128, KC*N]  fp8   xt[p, kc*N+n]   = x[n, kc*128+p]
           wt [128, KC*72] bf16  wt[p, kc*72+e]  = w_ext[kc*128+p, e]
                                 (w_ext = [W | W @ blockdiag-reduced a_dst])
           at [128, NJ*R]  fp8   at[p, nj*R+r]   = adj[c*R+r, nj*128+p]
  output:  out_p [128, RC*64] f32  out_p[p, q*64+f] = out[c*R+q*128+p, (h,f)]
"""

import sys

import numpy as np

if "/opt/trn_rl_repo" not in sys.path:
    sys.path.insert(0, "/opt/trn_rl_repo")

import ml_dtypes  # noqa: E402

import concourse.bass as bass  # noqa: E402
import concourse.tile as tile  # noqa: E402
from concourse import bacc, mybir  # noqa: E402
from concourse.bass_utils import run_bass_kernel_spmd  # noqa: E402
from concourse.masks import make_identity  # noqa: E402

N_CORES = 8
H = 8
F = 8
HF = H * F  # 64
EXT = HF + H  # 72: [g | z]
K_IN = 1024
P = 128

FP32 = mybir.dt.float32
BF16 = mybir.dt.bfloat16
FP8 = mybir.dt.float8e4
AFT = mybir.ActivationFunctionType
ALU = mybir.AluOpType


def _bcast_head(ap_ph):
    """[P, H] AP -> [P, H, F] AP broadcasting each head value over F."""
    return bass.AP(
        tensor=ap_ph.tensor,
        offset=ap_ph.offset,
        ap=[ap_ph.ap[0], ap_ph.ap[1], [0, F]],
    )


def build_bass(n_nodes: int) -> bass.Bass:
    R = n_nodes // N_CORES
    KC = K_IN // P  # k-chunks for the h matmul
    NJ = n_nodes // P  # j (neighbor) 128-chunks
    RC = R // P  # 128-row output chunks per core
    NHALF = n_nodes // 2  # node-axis halves for the h pipeline
    GH = NHALF // 512  # 512-col psum groups per half (4)
    QH = NHALF // P  # 128-node chunks per half (16)
    NJH = NJ // 2  # j-chunks per half (16)
    assert R % P == 0

    # Bacc finalize() runs move_matmul_waits_to_ldweights +
    # generate_event_semaphores, which legalize multi-wait instructions
    # for walrus (TRN2 allows at most 1 sync wait per instruction).
    nc = bacc.Bacc(num_devices=N_CORES)

    xt = nc.declare_dram_parameter("xt", [P, KC * n_nodes], FP8, isOutput=False)
    wt = nc.declare_dram_parameter("wt", [P, KC * EXT], BF16, isOutput=False)
    at = nc.declare_dram_parameter("at", [P, NJ * R], FP8, isOutput=False)
    out = nc.declare_dram_parameter("out", [P, RC * HF], FP32, isOutput=True)

    with tile.TileContext(nc) as tc:
        with (
            tc.tile_pool(name="singles", bufs=1) as singles,
            tc.tile_pool(name="xstream", bufs=4) as xstream,
            tc.tile_pool(name="bigpsum", bufs=4, space="PSUM") as bigpsum,
            tc.tile_pool(name="outpsum", bufs=1, space="PSUM") as outpsum,
            tc.tile_pool(name="smallpsum", bufs=3, space="PSUM") as smallpsum,
            tc.tile_pool(name="work", bufs=4) as work,
            tc.tile_pool(name="post", bufs=4) as post,
        ):
            ident = singles.tile([P, P], FP32)
            make_identity(nc, ident)

            # --- loads (p-major, one contiguous run per partition) ---
            w_sb = singles.tile([P, KC, EXT], BF16)
            nc.sync.dma_start(
                out=w_sb, in_=wt[:].rearrange("p (c e) -> p c e", c=KC)
            )
            xt_view = xt[:].rearrange("p (c n) -> p c n", c=KC)
            at_sb = singles.tile([P, NJ, R], FP8)
            at_view = at[:].rearrange("p (n r) -> p n r", n=NJ)

            hT_sb = singles.tile([EXT, n_nodes], FP32)
            g_sb = singles.tile([P, NJ, EXT], BF16)
            outT_ps = outpsum.tile([EXT, R], FP32)

            for half in range(2):
                noff = half * NHALF
                # --- stream xt, h matmuls kc-outer into GH psum banks ---
                hps = [
                    bigpsum.tile([EXT, 512], FP32, tag="bigps", name=f"h{half}_{g}")
                    for g in range(GH)
                ]
                for kc in range(KC):
                    xbuf = xstream.tile([P, NHALF], FP8, tag="xs")
                    nc.sync.dma_start(
                        out=xbuf, in_=xt_view[:, kc, noff : noff + NHALF]
                    )
                    for g in range(GH):
                        nc.tensor.matmul(
                            hps[g],
                            lhsT=w_sb[:, kc, :],
                            rhs=xbuf[:, g * 512 : (g + 1) * 512],
                            start=(kc == 0),
                            stop=(kc == KC - 1),
                        )
                # adjacency for this half's j-chunks follows xt on the queue
                for s in range(2):
                    lo = half * NJH + s * (NJH // 2)
                    hi = lo + NJH // 2
                    nc.sync.dma_start(
                        out=at_sb[:, lo:hi, :], in_=at_view[:, lo:hi, :]
                    )
                for g in range(GH):
                    nc.vector.tensor_copy(
                        hT_sb[:, noff + g * 512 : noff + (g + 1) * 512], hps[g]
                    )

                # --- per 128-node chunk: transpose, z=exp, G=[h*z | z] ---
                for q2 in range(QH):
                    q = half * QH + q2
                    h_ps = smallpsum.tile([P, EXT], FP32, tag="smallps")
                    nc.tensor.transpose(
                        h_ps, hT_sb[:, q * P : (q + 1) * P], ident[:EXT, :EXT]
                    )
                    z_sb = work.tile([P, H], FP32, tag="z")
                    nc.scalar.activation(z_sb, h_ps[:, HF:EXT], AFT.Exp)
                    nc.vector.tensor_mul(
                        g_sb[:, q, 0:HF].rearrange("p (h f) -> p h f", h=H),
                        h_ps[:, 0:HF].rearrange("p (h f) -> p h f", h=H),
                        _bcast_head(z_sb),
                    )
                    nc.vector.tensor_copy(g_sb[:, q, HF:EXT], z_sb)

                # --- aggregate this half's j-chunks: outT += G_n.T @ adjT_n ---
                for n2 in range(NJH):
                    n = half * NJH + n2
                    nc.tensor.matmul(
                        outT_ps,
                        lhsT=g_sb[:, n, :],
                        rhs=at_sb[:, n, :],
                        start=(n == 0),
                        stop=(n == NJ - 1),
                    )

            outT_sb = singles.tile([EXT, R], FP32)
            nc.vector.tensor_copy(outT_sb, outT_ps)

            # --- postprocess, batched per stage across the RC chunks ---
            o_ps = [None] * RC
            for q in range(RC):
                o_ps[q] = smallpsum.tile([P, EXT], FP32, tag="smallps", name=f"o{q}")
                nc.tensor.transpose(
                    o_ps[q], outT_sb[:, q * P : (q + 1) * P], ident[:EXT, :EXT]
                )
            xo = [None] * RC
            for q in range(RC):
                rd = work.tile([P, H], FP32, tag="rd")
                nc.vector.reciprocal(rd, o_ps[q][:, HF:EXT])
                xo[q] = post.tile([P, HF], FP32, tag="xo", name=f"xo{q}")
                nc.vector.tensor_mul(
                    xo[q].rearrange("p (h f) -> p h f", h=H),
                    o_ps[q][:, 0:HF].rearrange("p (h f) -> p h f", h=H),
                    _bcast_head(rd),
                )
            # y = relu(xo) + exp(min(xo, 0))  (= elu + 1; log_softmax shift-safe)
            yo = [None] * RC
            eo = [None] * RC
            for q in range(RC):
                mo = work.tile([P, HF], FP32, tag="mo")
                nc.vector.tensor_scalar_min(mo, xo[q], 0.0)
                eo[q] = post.tile([P, HF], FP32, tag="eo", name=f"eo{q}")
                nc.scalar.activation(eo[q], mo, AFT.Exp)
            for q in range(RC):
                yo[q] = post.tile([P, HF], FP32, tag="yo", name=f"yo{q}")
                nc.vector.scalar_tensor_tensor(
                    out=yo[q], in0=xo[q], scalar=0.0, in1=eo[q],
                    op0=ALU.max, op1=ALU.add,
                )
            # log-softmax over the 64 features (no max subtraction needed:
            # y in (0, ~10], exp stays in fp32 range). The Exp's accum_out
            # gives sum(exp) for free; one batched Ln covers all chunks.
            ex = [None] * RC
            sms = post.tile([P, RC], FP32, name="sms")
            for q in range(RC):
                ex[q] = post.tile([P, HF], FP32, tag="ex", name=f"ex{q}")
                nc.scalar.activation(
                    ex[q], yo[q], AFT.Exp, accum_out=sms[:, q : q + 1]
                )
            ls = post.tile([P, RC], FP32, name="ls")
            nc.scalar.activation(ls, sms, AFT.Ln)
            out_sb = singles.tile([P, RC, HF], FP32)
            for q in range(RC):
                nc.vector.tensor_scalar_sub(out_sb[:, q, :], yo[q], ls[:, q : q + 1])
            nc.sync.dma_start(out=out[:], in_=out_sb)

    # Force all ACT activations (Exp + Ln) onto the one table set containing
    # both, so only ONE ACT_TABLE_LOAD is emitted (early, hidden under DMA)
    # instead of a ~1.3us reload at every Exp<->Ln switch. Set indices must
    # stay aligned with act_info.json, so empty the other sets rather than
    # filtering the list.
    orig_gat = bacc.get_activation_tables

    def _one_set(arch):
        return {
            k: (v if k == "natural_log_exp_and_others" else set())
            for k, v in orig_gat(arch).items()
        }

    bacc.get_activation_tables = _one_set
    try:
        nc.finalize()
    finally:
        bacc.get_activation_tables = orig_gat
    return nc


def _host_prep(x, adj, W, a_dst, n_nodes):
    """Build per-core input maps."""
    R = n_nodes // N_CORES
    NJ = n_nodes // P
    KC = K_IN // P
    Wd = np.einsum(
        "khf,hf->kh", W.reshape(K_IN, H, F), a_dst, dtype=np.float32
    ).astype(np.float32)
    w_ext = np.concatenate([W, Wd], axis=1)  # [1024, 72] fp32
    # wt[p, kc*72+e] = w_ext[kc*128+p, e]
    wt = np.ascontiguousarray(
        w_ext.reshape(KC, P, EXT).transpose(1, 0, 2).reshape(P, KC * EXT)
    ).astype(ml_dtypes.bfloat16)
    # xt[p, kc*N+n] = x[n, kc*128+p]  (shared by all cores)
    xt = np.ascontiguousarray(
        x.astype(ml_dtypes.float8_e4m3).T.reshape(KC, P, n_nodes)
        .transpose(1, 0, 2).reshape(P, KC * n_nodes)
    )
    # adj as fp8e4: 0/1 exact. Byte-level build (1.0 == 0x38) beats .astype.
    adj_u8 = (adj.astype(np.uint8) * np.uint8(0x38)).view(ml_dtypes.float8_e4m3)
    in_maps = []
    for c in range(N_CORES):
        rows = slice(c * R, (c + 1) * R)
        # at[p, nj*R+r] = adj[c*R+r, nj*128+p]
        at = np.ascontiguousarray(
            adj_u8[rows].T.reshape(NJ, P, R).transpose(1, 0, 2).reshape(P, NJ * R)
        )
        in_maps.append({"xt": xt, "at": at, "wt": wt})
    return in_maps


_BUILT = {}


def run(x, adj, W, a_dst, trace=False):
    n_nodes = x.shape[0]
    R = n_nodes // N_CORES
    RC = R // P
    if n_nodes not in _BUILT:
        _BUILT[n_nodes] = build_bass(n_nodes)
    nc = _BUILT[n_nodes]
    in_maps = _host_prep(x, adj, W, a_dst, n_nodes)
    res = run_bass_kernel_spmd(
        nc, in_maps, list(range(N_CORES)), trace=trace
    )
    blocks = []
    for c in range(N_CORES):
        o = res.results[c]["out"]  # [P, RC*HF] p-major
        blocks.append(
            o.reshape(P, RC, HF).transpose(1, 0, 2).reshape(R, HF)
        )
    return np.concatenate(blocks, axis=0).astype(np.float32), res


def kernel(x, adj, W, a_src, a_dst):
    x = np.asarray(x, dtype=np.float32)
    adj = np.asarray(adj)
    W = np.asarray(W, dtype=np.float32)
    a_dst = np.asarray(a_dst, dtype=np.float32)
    out, _ = run(x, adj, W, a_dst, trace=False)
    return out


# revision 17
# speedup vs baseline: 2.3841x; 1.0906x over previous
"""GAT-style GNN message passing on 8 TRN2 NeuronCores — collective-free.

Math: with LEAK=1 the leaky-relu is identity, so
  e[i,j,h] = e_src[i,h] + e_dst[j,h]
and softmax over j cancels e_src (and any row max) exactly:
  attn[i,j,h] = adj[i,j]*exp(e_dst[j,h]) / sum_j adj[i,j]*exp(e_dst[j,h])
  out[i,(h,f)] = (adj @ (z*h))[i,(h,f)] / (adj @ z)[i,h],  z = exp(e_dst)
then elu + log_softmax per row. log_softmax is shift invariant, so
elu(x) is computed as relu(x) + exp(min(x,0)) (drops the uniform -1),
and no max subtraction is needed (y is bounded in (0, ~10]).

Sharding: rows (query nodes) of adj/out across 8 cores. The h = x@W
computation (cheap: 0.6 GFLOP) is REPLICATED on every core from a full
copy of x: zero collectives (an AllGather version paid a ~40us CC
barrier), zero cross-core dependencies, PE stays HAM-warm.

Dtypes (runs are HBM-bound, so bytes are everything): x fp8e4 (4MB
replicated), W bf16 (mixed bf16-stationary x fp8-moving matmuls are
legal on the PE, HW-verified), adjacency fp8e4 (0/1 exact, 2MB/core),
G=[z*h | z] bf16 stationary vs the fp8 moving adjT. fp32 PSUM accum.

Pipeline: x streams in GROUP-MAJOR layout — each 512KB transfer holds
ALL EIGHT k-chunks for one 512-node group, so the full chain
  h-matmuls -> evac -> 4x(transpose, z=exp, G=z*h) -> 4 agg matmuls
completes per group and runs concurrently with the next group's DMA.
Adjacency splits are interleaved (x0 a0 x1 x2 a1 x3 x4 a2 x5 x6 a3 x7)
so aggregation never waits on adj and the tail after the last byte is
just one group's chain + the postprocess.

Per-core device program (R = N/8 = 512 rows, P = 128, NG = 8 groups):
  inputs:  xt [128, NG*KC*512] fp8  xt[p, g*4096+kc*512+n] = x[g*512+n, kc*128+p]
           wt [128, KC*72] bf16     wt[p, kc*72+e] = w_ext[kc*128+p, e]
                                    (w_ext = [W | W @ blockdiag-reduced a_dst])
           at [128, NJ*R]  fp8      at[p, nj*R+r]  = adj[c*R+r, nj*128+p]
  output:  out_p [128, RC*64] f32   out_p[p, q*64+f] = out[c*R+q*128+p, (h,f)]
"""

import sys

import numpy as np

if "/opt/trn_rl_repo" not in sys.path:
    sys.path.insert(0, "/opt/trn_rl_repo")

import ml_dtypes  # noqa: E402

import concourse.bass as bass  # noqa: E402
import concourse.tile as tile  # noqa: E402
from concourse import bacc, mybir  # noqa: E402
from concourse.bass_utils import run_bass_kernel_spmd  # noqa: E402
from concourse.masks import make_identity  # noqa: E402

N_CORES = 8
H = 8
F = 8
HF = H * F  # 64
EXT = HF + H  # 72: [g | z]
K_IN = 1024
P = 128

FP32 = mybir.dt.float32
BF16 = mybir.dt.bfloat16
FP8 = mybir.dt.float8e4
AFT = mybir.ActivationFunctionType
ALU = mybir.AluOpType


def _bcast_head(ap_ph):
    """[P, H] AP -> [P, H, F] AP broadcasting each head value over F."""
    return bass.AP(
        tensor=ap_ph.tensor,
        offset=ap_ph.offset,
        ap=[ap_ph.ap[0], ap_ph.ap[1], [0, F]],
    )


def build_bass(n_nodes: int) -> bass.Bass:
    R = n_nodes // N_CORES
    KC = K_IN // P  # k-chunks for the h matmul
    NJ = n_nodes // P  # j (neighbor) 128-chunks
    NG = n_nodes // 512  # 512-node groups
    RC = R // P  # 128-row output chunks per core
    assert R % P == 0

    # Bacc finalize() runs move_matmul_waits_to_ldweights +
    # generate_event_semaphores, which legalize multi-wait instructions
    # for walrus (TRN2 allows at most 1 sync wait per instruction).
    nc = bacc.Bacc(num_devices=N_CORES)

    xt = nc.declare_dram_parameter("xt", [P, NG * KC * 512], FP8, isOutput=False)
    wt = nc.declare_dram_parameter("wt", [P, KC * EXT], BF16, isOutput=False)
    at = nc.declare_dram_parameter("at", [P, NJ * R], FP8, isOutput=False)
    out = nc.declare_dram_parameter("out", [P, RC * HF], FP32, isOutput=True)

    with tile.TileContext(nc) as tc:
        with (
            tc.tile_pool(name="singles", bufs=1) as singles,
            tc.tile_pool(name="xstream", bufs=3) as xstream,
            tc.tile_pool(name="hbuf", bufs=2) as hbuf,
            tc.tile_pool(name="hpsum", bufs=2, space="PSUM") as hpsum,
            tc.tile_pool(name="outpsum", bufs=1, space="PSUM") as outpsum,
            tc.tile_pool(name="smallpsum", bufs=3, space="PSUM") as smallpsum,
            tc.tile_pool(name="work", bufs=4) as work,
            tc.tile_pool(name="post", bufs=4) as post,
        ):
            ident = singles.tile([P, P], FP32)
            make_identity(nc, ident)

            # --- loads (p-major, one contiguous run per partition) ---
            w_sb = singles.tile([P, KC, EXT], BF16)
            nc.sync.dma_start(
                out=w_sb, in_=wt[:].rearrange("p (c e) -> p c e", c=KC)
            )
            xt_view = xt[:].rearrange("p (g c n) -> p g c n", g=NG, c=KC)
            at_sb = singles.tile([P, NJ, R], FP8)
            at_view = at[:].rearrange("p (n r) -> p n r", n=NJ)

            g_sb = singles.tile([P, NJ, EXT], BF16)
            outT_ps = outpsum.tile([EXT, R], FP32)

            # Streamed per-group pipeline. DMA order on the sync queue:
            # w x0 x1 a0 x2 x3 a1 x4 x5 a2 x6 a3 x7 out
            for g in range(NG):
                xbuf = xstream.tile([P, KC, 512], FP8, tag="xs")
                nc.sync.dma_start(out=xbuf, in_=xt_view[:, g])
                if g % 2 == 0:
                    # Each adjacency split MUST be emitted before the
                    # aggregation matmuls of the two groups it feeds:
                    # Tile's dependency tracking is trace-ordered, so a
                    # consumer emitted before its producer silently
                    # reads stale memory (and races the later DMA).
                    s = g // 2
                    nc.sync.dma_start(
                        out=at_sb[:, 8 * s : 8 * s + 8, :],
                        in_=at_view[:, 8 * s : 8 * s + 8, :],
                    )
                hg_ps = hpsum.tile([EXT, 512], FP32, tag="hps")
                for kc in range(KC):
                    nc.tensor.matmul(
                        hg_ps,
                        lhsT=w_sb[:, kc, :],
                        rhs=xbuf[:, kc, :],
                        start=(kc == 0),
                        stop=(kc == KC - 1),
                    )
                hb = hbuf.tile([EXT, 512], FP32, tag="hb")
                nc.vector.tensor_copy(hb, hg_ps)

                for q2 in range(4):
                    q = 4 * g + q2
                    h_ps = smallpsum.tile([P, EXT], FP32, tag="smallps")
                    nc.tensor.transpose(
                        h_ps, hb[:, q2 * P : (q2 + 1) * P], ident[:EXT, :EXT]
                    )
                    z_sb = work.tile([P, H], FP32, tag="z")
                    nc.scalar.activation(z_sb, h_ps[:, HF:EXT], AFT.Exp)
                    nc.vector.tensor_mul(
                        g_sb[:, q, 0:HF].rearrange("p (h f) -> p h f", h=H),
                        h_ps[:, 0:HF].rearrange("p (h f) -> p h f", h=H),
                        _bcast_head(z_sb),
                    )
                    nc.vector.tensor_copy(g_sb[:, q, HF:EXT], z_sb)

                # aggregate this group's j-chunks: outT += G_n.T @ adjT_n
                for n2 in range(4):
                    n = 4 * g + n2
                    nc.tensor.matmul(
                        outT_ps,
                        lhsT=g_sb[:, n, :],
                        rhs=at_sb[:, n, :],
                        start=(n == 0),
                        stop=(n == NJ - 1),
                    )

            outT_sb = singles.tile([EXT, R], FP32)
            nc.vector.tensor_copy(outT_sb, outT_ps)

            # --- postprocess, batched per stage across the RC chunks ---
            o_ps = [None] * RC
            for q in range(RC):
                o_ps[q] = smallpsum.tile([P, EXT], FP32, tag="smallps", name=f"o{q}")
                nc.tensor.transpose(
                    o_ps[q], outT_sb[:, q * P : (q + 1) * P], ident[:EXT, :EXT]
                )
            xo = [None] * RC
            for q in range(RC):
                rd = work.tile([P, H], FP32, tag="rd")
                nc.vector.reciprocal(rd, o_ps[q][:, HF:EXT])
                xo[q] = post.tile([P, HF], FP32, tag="xo", name=f"xo{q}")
                nc.vector.tensor_mul(
                    xo[q].rearrange("p (h f) -> p h f", h=H),
                    o_ps[q][:, 0:HF].rearrange("p (h f) -> p h f", h=H),
                    _bcast_head(rd),
                )
            # y = relu(xo) + exp(min(xo, 0))  (= elu + 1; log_softmax shift-safe)
            yo = [None] * RC
            eo = [None] * RC
            for q in range(RC):
                mo = work.tile([P, HF], FP32, tag="mo")
                nc.vector.tensor_scalar_min(mo, xo[q], 0.0)
                eo[q] = post.tile([P, HF], FP32, tag="eo", name=f"eo{q}")
                nc.scalar.activation(eo[q], mo, AFT.Exp)
            for q in range(RC):
                yo[q] = post.tile([P, HF], FP32, tag="yo", name=f"yo{q}")
                nc.vector.scalar_tensor_tensor(
                    out=yo[q], in0=xo[q], scalar=0.0, in1=eo[q],
                    op0=ALU.max, op1=ALU.add,
                )
            # log-softmax over the 64 features (no max subtraction needed:
            # y in (0, ~10], exp stays in fp32 range); batch Exp then Ln to
            # avoid ACT table-set thrash.
            ex = [None] * RC
            sm = [None] * RC
            for q in range(RC):
                ex[q] = post.tile([P, HF], FP32, tag="ex", name=f"ex{q}")
                nc.scalar.activation(ex[q], yo[q], AFT.Exp)
            for q in range(RC):
                sm[q] = post.tile([P, 1], FP32, tag="sm", name=f"sm{q}")
                nc.vector.reduce_sum(sm[q], ex[q], axis=mybir.AxisListType.X)
            out_sb = singles.tile([P, RC, HF], FP32)
            for q in range(RC):
                ls = work.tile([P, 1], FP32, tag="ls")
                nc.scalar.activation(ls, sm[q], AFT.Ln)
                nc.vector.tensor_scalar_sub(out_sb[:, q, :], yo[q], ls)
            nc.sync.dma_start(out=out[:], in_=out_sb)

    # Force all ACT activations (Exp + Ln) onto the one table set containing
    # both, so only ONE ACT_TABLE_LOAD is emitted (early, hidden under DMA)
    # instead of a ~1.3us reload at every Exp<->Ln switch. Set indices must
    # stay aligned with act_info.json, so empty the other sets rather than
    # filtering the list.
    orig_gat = bacc.get_activation_tables

    def _one_set(arch):
        return {
            k: (v if k == "natural_log_exp_and_others" else set())
            for k, v in orig_gat(arch).items()
        }

    bacc.get_activation_tables = _one_set
    try:
        nc.finalize()
    finally:
        bacc.get_activation_tables = orig_gat
    return nc


def _host_prep(x, adj, W, a_dst, n_nodes):
    """Build per-core input maps."""
    R = n_nodes // N_CORES
    NJ = n_nodes // P
    KC = K_IN // P
    Wd = np.einsum(
        "khf,hf->kh", W.reshape(K_IN, H, F), a_dst, dtype=np.float32
    ).astype(np.float32)
    w_ext = np.concatenate([W, Wd], axis=1)  # [1024, 72] fp32
    # wt[p, kc*72+e] = w_ext[kc*128+p, e]
    wt = np.ascontiguousarray(
        w_ext.reshape(KC, P, EXT).transpose(1, 0, 2).reshape(P, KC * EXT)
    ).astype(ml_dtypes.bfloat16)
    # xt[p, g*4096+kc*512+n2] = x[g*512+n2, kc*128+p]  (shared by all cores)
    xT = x.astype(ml_dtypes.float8_e4m3).T  # [1024, 4096]
    xt = np.ascontiguousarray(
        xT.reshape(KC, P, n_nodes // 512, 512).transpose(1, 2, 0, 3)
        .reshape(P, n_nodes * KC)
    )
    # adj as fp8e4: 0/1 exact (1.0 == 0x38); byte-level build beats .astype.
    adj_u8 = (adj.astype(np.uint8) * np.uint8(0x38)).view(ml_dtypes.float8_e4m3)
    in_maps = []
    for c in range(N_CORES):
        rows = slice(c * R, (c + 1) * R)
        # at[p, nj*R+r] = adj[c*R+r, nj*128+p]
        at = np.ascontiguousarray(
            adj_u8[rows].T.reshape(NJ, P, R).transpose(1, 0, 2).reshape(P, NJ * R)
        )
        in_maps.append({"xt": xt, "at": at, "wt": wt})
    return in_maps


_BUILT = {}


def run(x, adj, W, a_dst, trace=False):
    n_nodes = x.shape[0]
    R = n_nodes // N_CORES
    RC = R // P
    if n_nodes not in _BUILT:
        _BUILT[n_nodes] = build_bass(n_nodes)
    nc = _BUILT[n_nodes]
    in_maps = _host_prep(x, adj, W, a_dst, n_nodes)
    res = run_bass_kernel_spmd(
        nc, in_maps, list(range(N_CORES)), trace=trace
    )
    blocks = []
    for c in range(N_CORES):
        o = res.results[c]["out"]  # [P, RC*HF] p-major
        blocks.append(
            o.reshape(P, RC, HF).transpose(1, 0, 2).reshape(R, HF)
        )
    return np.concatenate(blocks, axis=0).astype(np.float32), res


def kernel(x, adj, W, a_src, a_dst):
    x = np.asarray(x, dtype=np.float32)
    adj = np.asarray(adj)
    W = np.asarray(W, dtype=np.float32)
    a_dst = np.asarray(a_dst, dtype=np.float32)
    out, _ = run(x, adj, W, a_dst, trace=False)
    return out
